# revision 13
# baseline (speedup 1.0000x reference)
"""Trainium2 Bass kernel for nn_BucketedGoWatti (sparse windowed attention pooling).

Math (B=4, L=4096, T=32, DH=1024, DG=256, DP=256, WIN=1024, STRIDE=256, W=13):
  q  = G @ Wq_core;  k = H @ Wk_core (window-independent)
  logits[b,w,t,l] = slice of global  s * (q @ Wk_core^T) @ H^T
  alpha = softmax in window; Zw[b,t,w,:] = alpha @ Hw
  wlog[b,t,w] = Zw . qw2,  qw2 = (G@Wq_win) @ Wk_win^T * DH^-0.5
  Z = softmax_w(wlog) @ Zw   (tiny; done on host at gather time)

Sharding: core c -> batch b=c//2, window half c%2 (even: windows 0-6 over
l in [0,2560); odd: windows 6-12 over l in [1536,4096); window 6 duplicated
so all 8 cores run one SPMD program shape). Cross-window combine on host.

Precision: big matmuls in float32r (~1.5e-4 rel); qw2 path bf16 (negligible
through the 13-way combine softmax); softmax/normalization fp32.
"""
import numpy as np
import ml_dtypes
from contextlib import ExitStack

import concourse.bacc as bacc
import concourse.tile as tile
import concourse.mybir as mybir
import concourse.masks as masks
from concourse.bass_utils import run_bass_kernel_spmd

F32 = mybir.dt.float32
F32R = mybir.dt.float32r
BF16 = mybir.dt.bfloat16
ActFn = mybir.ActivationFunctionType
Alu = mybir.AluOpType

B, L, T = 4, 4096, 32
DH, DG, DP = 1024, 256, 256
WIN, STRIDE = 1024, 256
W = (L - WIN) // STRIDE + 1          # 13
SPAN = 2560                          # per-core l-span
NLT = SPAN // 128                    # 20 l-tiles
NCH = SPAN // 256                    # 10 logits chunks of 256
WLOC = 7                             # windows per core
NDT = DH // 128                      # 8 d-tiles
S_CORE = 1.0 / float(np.sqrt(DP))
S_WIN = 1.0 / float(np.sqrt(DH))

_CACHE = {}


def _build(with_mask: bool, stage: int = 99):
    nc = bacc.Bacc("TRN2", debug=False, target_bir_lowering=False)

    Hn_d = nc.dram_tensor("Hn", [SPAN, DH], F32R, kind="ExternalInput")
    HT_d = nc.dram_tensor("HT", [DH, SPAN], F32R, kind="ExternalInput")
    GT_d = nc.dram_tensor("GT", [DG, T], F32R, kind="ExternalInput")
    Wqc_d = nc.dram_tensor("Wqc", [DG, DP], F32R, kind="ExternalInput")
    WkcT_d = nc.dram_tensor("WkcT", [DP, DH], F32R, kind="ExternalInput")
    Wqw_d = nc.dram_tensor("Wqw", [DG, DH], F32R, kind="ExternalInput")
    WkwT_d = nc.dram_tensor("WkwT", [DH, DH], BF16, kind="ExternalInput")
    if with_mask:
        mb_d = nc.dram_tensor("maskbias", [1, SPAN], F32R, kind="ExternalInput")
        ones_d = nc.dram_tensor("onesrow", [1, T], F32R, kind="ExternalInput")
    zw_d = nc.dram_tensor("Zw_out", [WLOC * T, DH], F32, kind="ExternalOutput")
    wl_d = nc.dram_tensor("wlog_out", [T, WLOC], F32, kind="ExternalOutput")

    with tile.TileContext(nc) as tc, ExitStack() as ctx:
        const = ctx.enter_context(tc.tile_pool(name="const", bufs=1))
        hpool = ctx.enter_context(tc.tile_pool(name="hpool", bufs=14))
        htp = ctx.enter_context(tc.tile_pool(name="htp", bufs=10))
        sb = ctx.enter_context(tc.tile_pool(name="sb", bufs=1))
        sexp = ctx.enter_context(tc.tile_pool(name="sexp", bufs=1))
        pj = ctx.enter_context(tc.tile_pool(name="pj", bufs=2, space="PSUM"))
        lg = ctx.enter_context(tc.tile_pool(name="lg", bufs=2, space="PSUM"))
        zp = ctx.enter_context(tc.tile_pool(name="zp", bufs=4, space="PSUM"))

        # ---- small resident inputs ----
        ident = const.tile([128, 128], F32, tag="ident")
        masks.make_identity(nc, ident[:])
        gt = const.tile([128, 2 * T], F32R, tag="gt")
        wqc = const.tile([128, 2 * DP], F32R, tag="wqc")
        wkcT = const.tile([128, 2 * DH], F32R, tag="wkcT")
        wqw = const.tile([128, 2 * DH], F32R, tag="wqw")
        wkwT = const.tile([128, NDT * DH], BF16, tag="wkwT")
        for g in range(2):
            nc.gpsimd.dma_start(gt[:, g * T:(g + 1) * T], GT_d.ap()[g * 128:(g + 1) * 128, :])
            nc.gpsimd.dma_start(wqc[:, g * DP:(g + 1) * DP], Wqc_d.ap()[g * 128:(g + 1) * 128, :])
            nc.gpsimd.dma_start(wkcT[:, g * DH:(g + 1) * DH], WkcT_d.ap()[g * 128:(g + 1) * 128, :])
            nc.gpsimd.dma_start(wqw[:, g * DH:(g + 1) * DH], Wqw_d.ap()[g * 128:(g + 1) * 128, :])

        if with_mask:
            mbias = const.tile([1, SPAN], F32R, tag="mbias")
            onesr = const.tile([1, T], F32R, tag="onesr")
            nc.gpsimd.dma_start(mbias[:], mb_d.ap())
            nc.gpsimd.dma_start(onesr[:], ones_d.ap())

        # ---- q^T then qk^T ----
        qT = []
        for p in range(2):
            ps_ = pj.tile([128, 512], F32, tag="pj")
            for g in range(2):
                nc.tensor.matmul(ps_[:, :T], wqc[:, g * DP + p * 128:g * DP + (p + 1) * 128],
                                 gt[:, g * T:(g + 1) * T], start=(g == 0), stop=(g == 1))
            t_ = sb.tile([128, T], F32R, tag=f"qT{p}")
            nc.scalar.activation(t_[:], ps_[:, :T], ActFn.Identity, scale=S_CORE)
            qT.append(t_)
        qkT = []
        for i in range(NDT):
            ps_ = pj.tile([128, 512], F32, tag="pj")
            for p in range(2):
                nc.tensor.matmul(ps_[:, :T], wkcT[:, p * DH + i * 128:p * DH + (i + 1) * 128],
                                 qT[p][:], start=(p == 0), stop=(p == 1))
            t_ = sb.tile([128, T], F32R, tag=f"qkT{i}")
            nc.vector.tensor_copy(t_[:], ps_[:, :T])
            qkT.append(t_)
        if stage == 1:
            dbg = sb.tile([128, NDT * T], F32, tag="dbg")
            for i in range(NDT):
                nc.vector.tensor_copy(dbg[:, i * T:(i + 1) * T], qkT[i][:].bitcast(F32))
            nc.sync.dma_start(zw_d.ap()[:128, :NDT * T], dbg[:])

        # ---- logits chunks + exp (+ per-chunk sums) ----
        hn = []
        if stage >= 2:
            expL = sexp.tile([T, SPAN], F32, tag="expL")
            csum = sexp.tile([T, NCH], F32, tag="csum")
            ht = {}
            for half in range(2):
                for i in range(NDT):
                    t_ = htp.tile([128, SPAN // 2], F32R, tag="ht")
                    nc.sync.dma_start(t_[:], HT_d.ap()[i * 128:(i + 1) * 128,
                                                       half * (SPAN // 2):(half + 1) * (SPAN // 2)])
                    ht[(half, i)] = t_
            if stage >= 5:
                for j in range(NLT):
                    t_ = hpool.tile([128, DH], F32R, tag="hn")
                    nc.scalar.dma_start(t_[:], Hn_d.ap()[j * 128:(j + 1) * 128, :])
                    hn.append(t_)
            for e in range(NDT):
                nc.gpsimd.dma_start(wkwT[:, e * DH:(e + 1) * DH],
                                    WkwT_d.ap()[e * 128:(e + 1) * 128, :])
            for half in range(2):
                for off, width in ((0, 512), (512, 512), (1024, 256)):
                    gl = half * (SPAN // 2) + off       # global column offset
                    ps_ = lg.tile([T, 512], F32, tag="lg")
                    for i in range(NDT):
                        nc.tensor.matmul(ps_[:, :width], qkT[i][:], ht[(half, i)][:, off:off + width],
                                         start=(i == 0), stop=(i == NDT - 1 and not with_mask))
                    if with_mask:
                        nc.tensor.matmul(ps_[:, :width], onesr[:], mbias[:, gl:gl + width],
                                         start=False, stop=True)
                    for u in range(width // 256):
                        c = (gl + u * 256) // 256
                        nc.scalar.activation(expL[:, c * 256:(c + 1) * 256],
                                             ps_[:, u * 256:(u + 1) * 256],
                                             ActFn.Exp, accum_out=csum[:, c:c + 1])
            if stage == 2:
                nc.sync.dma_start(zw_d.ap()[:T, :DH], expL[:, :DH])

        # ---- transpose expL into [l, t] f32r tiles; denominators ----
        if stage >= 3:
            expLT = []
            for j in range(NLT):
                ps_ = pj.tile([128, 512], F32, tag="pj")
                nc.tensor.transpose(ps_[:, :T], expL[:, j * 128:(j + 1) * 128], ident[:T, :T])
                t_ = sb.tile([128, T], F32R, tag=f"eT{j}")
                nc.vector.tensor_copy(t_[:], ps_[:, :T])
                expLT.append(t_)
            dens = sexp.tile([T, WLOC], F32, tag="dens")
            recips = sexp.tile([T, WLOC], F32, tag="recips")
            for j in range(WLOC):
                nc.vector.reduce_sum(dens[:, j:j + 1], csum[:, j:j + 4], axis=mybir.AxisListType.X)
                nc.vector.reciprocal(recips[:, j:j + 1], dens[:, j:j + 1])
            if stage == 3:
                dbg = sb.tile([128, 2 * T], F32, tag="dbg")
                nc.vector.tensor_copy(dbg[:, :T], expLT[0][:].bitcast(F32))
                nc.vector.tensor_copy(dbg[:, T:2 * T], expLT[1][:].bitcast(F32))
                nc.sync.dma_start(zw_d.ap()[:128, :2 * T], dbg[:])
                nc.sync.dma_start(wl_d.ap(), recips[:])

        # ---- qw -> qw^T(bf16) -> qw2 ----
        if stage >= 4:
            qw = sb.tile([T, DH], F32, tag="qw")
            for h in range(2):
                ps_ = zp.tile([T, 512], F32, tag="zp")
                for g in range(2):
                    nc.tensor.matmul(ps_[:], gt[:, g * T:(g + 1) * T],
                                     wqw[:, g * DH + h * 512:g * DH + (h + 1) * 512],
                                     start=(g == 0), stop=(g == 1))
                nc.scalar.activation(qw[:, h * 512:(h + 1) * 512], ps_[:], ActFn.Identity,
                                     scale=S_WIN)
            qwT = []
            for e in range(NDT):
                ps_ = pj.tile([128, 512], F32, tag="pj")
                nc.tensor.transpose(ps_[:, :T], qw[:, e * 128:(e + 1) * 128], ident[:T, :T])
                t_ = sb.tile([128, T], BF16, tag=f"qwT{e}")
                nc.vector.tensor_copy(t_[:], ps_[:, :T])
                qwT.append(t_)
            qw2 = sb.tile([T, DH], F32, tag="qw2")
            for h in range(2):
                ps_ = zp.tile([T, 512], F32, tag="zp")
                for e in range(NDT):
                    nc.tensor.matmul(ps_[:], qwT[e][:],
                                     wkwT[:, e * DH + h * 512:e * DH + (h + 1) * 512],
                                     start=(e == 0), stop=(e == NDT - 1))
                nc.scalar.activation(qw2[:, h * 512:(h + 1) * 512], ps_[:], ActFn.Identity)
            if stage == 4:
                nc.sync.dma_start(zw_d.ap()[:T, :DH], qw2[:])

        # ---- Zw per window (normalized in PSUM->SBUF copy), wlog inline ----
        if stage >= 5:
            wlog = sexp.tile([T, WLOC], F32, tag="wlog")
            scratch = sexp.tile([T, DH], F32, tag="scratch")
            for j in range(WLOC):
                t_ = sb.tile([T, DH], F32, tag="zw")
                ps_a = zp.tile([T, 512], F32, tag="zp")
                ps_b = zp.tile([T, 512], F32, tag="zp")
                pss = [ps_a, ps_b]
                for k in range(8):
                    for h in range(2):
                        nc.tensor.matmul(pss[h][:], expLT[2 * j + k][:],
                                         hn[2 * j + k][:, h * 512:(h + 1) * 512],
                                         start=(k == 0), stop=(k == 7))
                for h in range(2):
                    nc.vector.tensor_scalar_mul(t_[:, h * 512:(h + 1) * 512], pss[h][:],
                                                recips[:, j:j + 1])
                nc.sync.dma_start(zw_d.ap()[j * T:(j + 1) * T, :], t_[:])
                if stage >= 7:
                    nc.vector.tensor_mul(scratch[:], t_[:], qw2[:])
                    nc.vector.reduce_sum(wlog[:, j:j + 1], scratch[:],
                                         axis=mybir.AxisListType.X)
            if stage >= 7:
                nc.gpsimd.dma_start(wl_d.ap(), wlog[:])

    nc.compile()
    return nc


def kernel(H, G, Wq_core, Wk_core, Wq_win, Wk_win, attn_mask):
    H = np.asarray(H, dtype=np.float32)
    G = np.asarray(G, dtype=np.float32)
    Wq_core = np.asarray(Wq_core, dtype=np.float32)
    Wk_core = np.asarray(Wk_core, dtype=np.float32)
    Wq_win = np.asarray(Wq_win, dtype=np.float32)
    Wk_win = np.asarray(Wk_win, dtype=np.float32)
    mask = np.asarray(attn_mask).astype(bool)

    with_mask = not bool(mask.all())
    key = ("k", with_mask)
    if key not in _CACHE:
        _CACHE[key] = _build(with_mask)
    nc = _CACHE[key]

    WkcT = np.ascontiguousarray(Wk_core.T)
    WkwT = np.ascontiguousarray(Wk_win.T).astype(ml_dtypes.bfloat16)

    in_maps = []
    for c in range(8):
        b, half = c // 2, c % 2
        lo = 0 if half == 0 else L - SPAN
        im = {
            "Hn": np.ascontiguousarray(H[b, lo:lo + SPAN, :]),
            "HT": np.ascontiguousarray(H[b].T[:, lo:lo + SPAN]),
            "GT": np.ascontiguousarray(G[b].T),
            "Wqc": Wq_core,
            "WkcT": WkcT,
            "Wqw": Wq_win,
            "WkwT": WkwT,
        }
        if with_mask:
            im["maskbias"] = np.where(mask[b, lo:lo + SPAN], 0.0, -1e9).astype(np.float32)[None, :]
            im["onesrow"] = np.ones((1, T), dtype=np.float32)
        in_maps.append(im)

    import os
    prof_dir = os.environ.get("BGW_PROFILE_DIR")
    if prof_dir:
        res = run_bass_kernel_spmd(nc, in_maps, core_ids=list(range(8)),
                                   trace=True, tmpdir=prof_dir)
    else:
        res = run_bass_kernel_spmd(nc, in_maps, core_ids=list(range(8)))
    kernel._last_result = res

    # ---- host combine: tiny cross-window softmax over W=13 ----
    Z = np.empty((B, T, DH), dtype=np.float32)
    for b in range(B):
        zw_full = np.empty((W, T, DH), dtype=np.float32)
        wl_full = np.empty((T, W), dtype=np.float32)
        for half in range(2):
            r = res.results[2 * b + half]
            zw = r["Zw_out"].reshape(WLOC, T, DH)
            wl = r["wlog_out"]
            w0 = 0 if half == 0 else W - WLOC
            zw_full[w0:w0 + WLOC] = zw
            wl_full[:, w0:w0 + WLOC] = wl
        m = wl_full.max(axis=1, keepdims=True)
        e = np.exp(wl_full - m)
        wsm = e / e.sum(axis=1, keepdims=True)          # [T, W]
        Z[b] = np.einsum("tw,wtd->td", wsm, zw_full)
    return Z
